# revision 1
# baseline (speedup 1.0000x reference)
"""Multi-head dot-product attention (with per-head LayerNorm on q/k/v) on 8
Trainium2 NeuronCores.

Model: x[4, 2048, 1024], 16 heads x 64 dim, LN (no affine) applied per head to
q/k/v projections, softmax attention, output projection.

Sharding: core = (batch, query-half). Each core owns one batch and 1024 query
tokens; it computes k/v for the full 2048 keys of its batch (25% duplicated
work, zero collectives). Attention is invariant to key order, so the host
rotates tokens per core to make the program pure SPMD (queries are always
rows 0:1024 of the per-core input).

Device layout highlights:
 - host pre-transposes x to xT [dmodel, seq], casts matmul operands to bf16,
   and augments the q/k/v weights with a bias row (ones row in xT) and 16
   per-head mean columns, so the projection matmul emits y+bias and its
   per-head means in one pass (no separate bias-add / mean-reduce on DVE)
 - LayerNorm: center directly while draining PSUM, variance from the centered
   values, rsqrt via DVE Newton iteration - ScalarE runs only exp (one ACT
   table set, no ~2.7us table reloads), so attention overlaps projections
 - normalized q/k DMA-transposed (XBAR, one 3D-output descriptor per tile)
   into [head_dim, token] layout for the attention matmuls
 - scores computed as sT [key, query]; no max-subtraction needed (LN bounds
   scores to +-8); softmax denominator via a ones-column appended to v
   (pv matmul M=65); 1/l broadcast via a DRAM-bounce DMA (partition-step-0
   APs are only legal on DRAM sources) and applied in the PSUM-drain multiply
"""

import sys

for _p in ("/opt/trn_rl_repo",):
    if _p not in sys.path:
        sys.path.insert(0, _p)

import numpy as np
import ml_dtypes
from contextlib import ExitStack

import concourse.bass as bass
import concourse.bacc as bacc
import concourse.tile as tile
from concourse import mybir
from concourse import bass_utils

BF16 = ml_dtypes.bfloat16

B, S, DM = 4, 2048, 1024
H, HD = 16, 64
NCORES = 8
SQ = S // 2          # query tokens per core
NT_K = S // 128      # 16 token tiles for k/v
NT_Q = SQ // 128     # 8 token tiles for q
NIT = DM // 128      # 8 contraction tiles
NOC = DM // 512      # 2 output column chunks
QB = 512             # query block width in attention
NQB = SQ // QB       # 2
LN_EPS = 1e-5


def _build_program():
    nc = bacc.Bacc("TRN2", target_bir_lowering=False, debug=False)

    f32 = mybir.dt.float32
    bf16 = mybir.dt.bfloat16

    # xt row 1024 is all-ones (bias fold); wq/wk/wv are host-augmented:
    # [1025, 1040] = rows [W; b], cols [W | W@M] with M the per-head mean
    # operator, so the projection matmul emits y+bias AND its per-head means.
    xT_d = nc.dram_tensor("xt", [DM + 1, S], bf16, kind="ExternalInput").ap()
    w_d = {
        n: nc.dram_tensor(f"w{n}", [DM + 1, DM + H], bf16, kind="ExternalInput").ap()
        for n in ("q", "k", "v")
    }
    w_d["o"] = nc.dram_tensor("wo", [DM, DM], bf16, kind="ExternalInput").ap()
    bo_d = nc.dram_tensor("bo", [1, DM], f32, kind="ExternalInput").ap()
    out_d = nc.dram_tensor("out", [SQ, DM], f32, kind="ExternalOutput").ap()

    with ExitStack() as ctx:
        tc = ctx.enter_context(tile.TileContext(nc))

        consts = ctx.enter_context(tc.tile_pool(name="consts", bufs=1))
        xT_p = ctx.enter_context(tc.tile_pool(name="xT", bufs=1))
        w_p = ctx.enter_context(tc.tile_pool(name="w", bufs=1))
        qT_p = ctx.enter_context(tc.tile_pool(name="qT", bufs=1))
        kT_p = ctx.enter_context(tc.tile_pool(name="kT", bufs=1))
        vA_p = ctx.enter_context(tc.tile_pool(name="vA", bufs=1))
        aT_p = ctx.enter_context(tc.tile_pool(name="aT", bufs=1))
        stage_p = ctx.enter_context(tc.tile_pool(name="stage", bufs=2))
        stagebf_p = ctx.enter_context(tc.tile_pool(name="stagebf", bufs=3))
        sq_p = ctx.enter_context(tc.tile_pool(name="sq", bufs=2))
        stats_p = ctx.enter_context(tc.tile_pool(name="stats", bufs=4))
        probs_p = ctx.enter_context(tc.tile_pool(name="probs", bufs=6))
        rr_p = ctx.enter_context(tc.tile_pool(name="rr", bufs=2))
        rb_p = ctx.enter_context(tc.tile_pool(name="rb", bufs=2))
        outst_p = ctx.enter_context(tc.tile_pool(name="outst", bufs=2))

        psA = ctx.enter_context(tc.tile_pool(name="psA", bufs=2, space="PSUM"))
        psM = ctx.enter_context(tc.tile_pool(name="psM", bufs=1, space="PSUM"))
        psS = ctx.enter_context(tc.tile_pool(name="psS", bufs=3, space="PSUM"))
        psO = ctx.enter_context(tc.tile_pool(name="psO", bufs=2, space="PSUM"))
        dram_p = ctx.enter_context(tc.tile_pool(name="dram", bufs=4, space="DRAM"))

        # ---- persistent tiles ----
        xT = xT_p.tile([128, NIT, S], bf16)
        nc.sync.dma_start(
            out=xT, in_=xT_d[0:DM, :].rearrange("(t p) s -> p t s", p=128)
        )
        xone = consts.tile([1, S], bf16, tag="xone")
        nc.sync.dma_start(out=xone, in_=xT_d[DM:DM + 1, :])

        bias_o = consts.tile([128, DM], f32, tag="bias_o")
        nc.gpsimd.dma_start(
            out=bias_o,
            in_=bass.AP(tensor=bo_d.tensor, offset=bo_d.offset,
                        ap=[[0, 128], bo_d.ap[1]]),
        )

        qT = qT_p.tile([128, NIT, SQ], bf16)    # [d-part, head-pair, q-token]
        kT = kT_p.tile([128, NIT, S], bf16)     # [d-part, head-pair, k-token]
        vA = vA_p.tile([128, NT_K, H, HD + 1], bf16)  # [k-part, ktile, head, d+1]
        aT = aT_p.tile([128, NIT, SQ], bf16)    # attn outT [d-part, head-pair, q]

        # ones column of v (softmax denominator rides along the pv matmul)
        nc.vector.memset(vA[:, :, :, HD:HD + 1], 1.0)

        i32 = mybir.dt.int32
        magic_t = consts.tile([128, H], i32, tag="magic")
        nc.vector.memset(magic_t, 0x5f3759df)

        # ---- projections + LN (+ transpose for q/k) ----
        def bcast3(t):
            return bass.AP(
                tensor=t.tensor, offset=t.offset,
                ap=[t.ap[0], t.ap[1], [0, HD]],
            )

        def load_w(name):
            ncols = DM if name == "o" else DM + H
            wt = w_p.tile([128, NIT, DM + H], bf16, tag="w")
            nc.sync.dma_start(
                out=wt[:, :, 0:ncols],
                in_=w_d[name][0:DM, :].rearrange("(t p) o -> p t o", p=128),
            )
            if name == "o":
                return wt, None
            wb = consts.tile([1, DM + H], bf16, tag=f"wb_{name}")
            nc.sync.dma_start(out=wb, in_=w_d[name][DM:DM + 1, :])
            return wt, wb

        def proj_ln(name, ntt):
            """Projection with bias+mean folded into the matmul, then LN."""
            wt, wb = load_w(name)
            for tt in range(ntt):
                tsl = slice(tt * 128, (tt + 1) * 128)
                # per-head means (+bias mean) straight from the PE
                pm = psM.tile([128, H], f32, tag="psM")
                for it in range(NIT):
                    nc.tensor.matmul(
                        pm, xT[:, it, tsl], wt[:, it, DM:DM + H],
                        start=(it == 0), stop=False,
                    )
                nc.tensor.matmul(
                    pm, xone[:, tsl], wb[:, DM:DM + H],
                    start=False, stop=True,
                )
                mu = stats_p.tile([128, H], f32, tag="mu")
                nc.vector.tensor_copy(out=mu, in_=pm)

                cen = stage_p.tile([128, DM], f32, tag="cen")
                cen3 = cen.rearrange("p (h d) -> p h d", h=H)
                for oc in range(NOC):
                    ps = psA.tile([128, 512], f32, tag="psA")
                    for it in range(NIT):
                        nc.tensor.matmul(
                            ps,
                            xT[:, it, tsl],
                            wt[:, it, oc * 512:(oc + 1) * 512],
                            start=(it == 0), stop=False,
                        )
                    nc.tensor.matmul(
                        ps, xone[:, tsl], wb[:, oc * 512:(oc + 1) * 512],
                        start=False, stop=True,
                    )
                    # drain+center in one op: cen = (y+b) - mu
                    nc.vector.tensor_sub(
                        out=cen3[:, oc * (NOC * 4):(oc + 1) * (NOC * 4), :],
                        in0=ps.rearrange("p (h d) -> p h d", h=512 // HD),
                        in1=bcast3(mu)[:, oc * (NOC * 4):(oc + 1) * (NOC * 4), :],
                    )
                # variance directly from centered values
                sqt = sq_p.tile([128, DM], f32, tag="sq")
                nc.vector.tensor_mul(out=sqt, in0=cen, in1=cen)
                ssq = stats_p.tile([128, H], f32, tag="ssq")
                nc.vector.tensor_reduce(
                    out=ssq, in_=sqt.rearrange("p (h d) -> p h d", h=H),
                    axis=mybir.AxisListType.X, op=mybir.AluOpType.add,
                )
                var = stats_p.tile([128, H], f32, tag="var")
                nc.vector.tensor_scalar(
                    out=var, in0=ssq, scalar1=1.0 / HD, scalar2=LN_EPS,
                    op0=mybir.AluOpType.mult, op1=mybir.AluOpType.add,
                )
                # rstd = rsqrt(var+eps): DVE Newton (no ACT sqrt -> ScalarE
                # runs exp only, one table set, attention overlaps freely)
                shi = stats_p.tile([128, H], i32, tag="shi")
                nc.vector.tensor_scalar(
                    out=shi, in0=var.bitcast(i32), scalar1=1, scalar2=None,
                    op0=mybir.AluOpType.logical_shift_right,
                )
                rstd = stats_p.tile([128, H], f32, tag="rstd")
                nc.vector.tensor_sub(
                    out=rstd.bitcast(i32), in0=magic_t, in1=shi)
                nt = stats_p.tile([128, H], f32, tag="nt")
                for _ in range(3):
                    nc.vector.tensor_mul(out=nt, in0=rstd, in1=rstd)
                    nc.vector.tensor_mul(out=nt, in0=nt, in1=var)
                    nc.vector.tensor_scalar(
                        out=nt, in0=nt, scalar1=-0.5, scalar2=1.5,
                        op0=mybir.AluOpType.mult, op1=mybir.AluOpType.add,
                    )
                    nc.vector.tensor_mul(out=rstd, in0=rstd, in1=nt)

                if name == "v":
                    nc.vector.tensor_mul(
                        out=vA[:, tt, :, 0:HD], in0=cen3, in1=bcast3(rstd),
                    )
                else:
                    nb = stagebf_p.tile([128, DM], bf16, tag="nbf")
                    nc.vector.tensor_mul(
                        out=nb.rearrange("p (h d) -> p h d", h=H),
                        in0=cen3, in1=bcast3(rstd),
                    )
                    dst = qT if name == "q" else kT
                    # one XBAR transpose for all 8 column blocks:
                    # dst[p, j, t] = nb[t, j*128+p]
                    nc.sync.dma_start_transpose(dst[:, :, tsl], nb)

        proj_ln("q", NT_Q)
        proj_ln("k", NT_K)
        proj_ln("v", NT_K)

        # ---- attention: qb outer so the out-projection of finished query
        # blocks overlaps the remaining attention work ----
        for qb in range(NQB):
            for j in range(NIT):       # head pair (heads 2j, 2j+1)
                qsl = slice(qb * QB, (qb + 1) * QB)
                oP = [
                    psO.tile([HD + 1, QB], f32, tag="psO", name=f"oP{hh}")
                    for hh in range(2)
                ]
                for kt in range(NT_K):
                    ksl = slice(kt * 128, (kt + 1) * 128)
                    for hh in range(2):
                        psl = slice(hh * HD, (hh + 1) * HD)
                        sp = psS.tile([128, QB], f32, tag="psS")
                        nc.tensor.matmul(
                            sp, kT[psl, j, ksl], qT[psl, j, qsl],
                            start=True, stop=True,
                        )
                        pt = probs_p.tile([128, QB], bf16, tag="probs")
                        nc.scalar.activation(
                            out=pt, in_=sp,
                            func=mybir.ActivationFunctionType.Exp,
                            scale=1.0 / np.sqrt(HD),
                        )
                        nc.tensor.matmul(
                            oP[hh], vA[:, kt, 2 * j + hh, :], pt,
                            start=(kt == 0), stop=(kt == NT_K - 1),
                        )
                for hh in range(2):
                    rt = rr_p.tile([1, QB], f32, tag="rr")
                    nc.vector.reciprocal(out=rt, in_=oP[hh][HD:HD + 1, :])
                    # broadcast r along partitions via a DRAM bounce (DRAM
                    # source APs may have partition step 0; SBUF may not)
                    rd = dram_p.tile([1, QB], f32, tag="rd")
                    nc.sync.dma_start(out=rd, in_=rt)
                    rbt = rb_p.tile([HD, QB], f32, tag="rb")
                    nc.sync.dma_start(
                        out=rbt,
                        in_=bass.AP(tensor=rd.tensor, offset=rd.offset,
                                    ap=[[0, HD], rd.ap[1]]),
                    )
                    nc.vector.tensor_mul(
                        out=aT[hh * HD:(hh + 1) * HD, j, qsl],
                        in0=oP[hh][0:HD, :], in1=rbt,
                    )

        # ---- output projection ----
        wo, _ = load_w("o")
        for tt in range(NT_Q):
            ot = outst_p.tile([128, DM], f32, tag="outst")
            for oc in range(NOC):
                ps = psA.tile([128, 512], f32, tag="psA")
                for j in range(NIT):
                    nc.tensor.matmul(
                        ps,
                        aT[:, j, tt * 128:(tt + 1) * 128],
                        wo[:, j, oc * 512:(oc + 1) * 512],
                        start=(j == 0), stop=(j == NIT - 1),
                    )
                nc.vector.tensor_add(
                    out=ot[:, oc * 512:(oc + 1) * 512],
                    in0=ps,
                    in1=bias_o[:, oc * 512:(oc + 1) * 512],
                )
            nc.sync.dma_start(out=out_d[tt * 128:(tt + 1) * 128, :], in_=ot)

    nc.compile()
    return nc


_CACHE = {}


def _get_program():
    if "nc" not in _CACHE:
        _CACHE["nc"] = _build_program()
    return _CACHE["nc"]


def _augment(W, b):
    """[W | W@M ; b | b@M] — M averages each head's 64 columns."""
    W = np.asarray(W, dtype=np.float32)
    b = np.asarray(b, dtype=np.float32)
    Wm = W.reshape(DM, H, HD).mean(axis=2)          # [DM, H]
    bm = b.reshape(H, HD).mean(axis=1)              # [H]
    top = np.concatenate([W, Wm], axis=1)           # [DM, DM+H]
    bot = np.concatenate([b, bm])[None, :]          # [1, DM+H]
    return np.ascontiguousarray(
        np.concatenate([top, bot], axis=0).astype(BF16))


def _make_in_maps(x, Wq, bq, Wk, bk, Wv, bv, Wo, bo):
    wq = _augment(Wq, bq)
    wk = _augment(Wk, bk)
    wv = _augment(Wv, bv)
    wo = np.ascontiguousarray(np.asarray(Wo).astype(BF16))
    bo_a = np.ascontiguousarray(np.asarray(bo, dtype=np.float32).reshape(1, DM))
    ones = np.ones((1, S), dtype=np.float32)
    in_maps = []
    for c in range(NCORES):
        b, hf = divmod(c, 2)
        xb = np.asarray(x[b])
        if hf:
            xb = np.concatenate([xb[SQ:], xb[:SQ]], axis=0)
        xt = np.ascontiguousarray(
            np.concatenate([xb.T, ones], axis=0).astype(BF16))
        in_maps.append({
            "xt": xt, "wq": wq, "wk": wk, "wv": wv, "wo": wo, "bo": bo_a,
        })
    return in_maps


def _run(x, Wq, bq, Wk, bk, Wv, bv, Wo, bo, **run_kwargs):
    nc = _get_program()
    in_maps = _make_in_maps(x, Wq, bq, Wk, bk, Wv, bv, Wo, bo)
    res = bass_utils.run_bass_kernel_spmd(
        nc, in_maps, core_ids=list(range(NCORES)), **run_kwargs
    )
    out = np.empty((B, S, DM), dtype=np.float32)
    for c in range(NCORES):
        b, hf = divmod(c, 2)
        out[b, hf * SQ:(hf + 1) * SQ] = res.results[c]["out"]
    return out, res


def kernel(x, Wq, bq, Wk, bk, Wv, bv, Wo, bo):
    out, _ = _run(x, Wq, bq, Wk, bk, Wv, bv, Wo, bo)
    return out


def kernel_profiled(x, Wq, bq, Wk, bk, Wv, bv, Wo, bo):
    return _run(x, Wq, bq, Wk, bk, Wv, bv, Wo, bo, trace=True)

